# revision 25
# baseline (speedup 1.0000x reference)
"""GAT layer kernel for 8 Trainium2 NeuronCores.

Math (per core, rows i in its 512-row slice, j = all 4096 nodes):
  g = x @ W1 -> [N, H, F];  el/er = head-wise projections of g on attn_l/attn_r
  e_ij = leaky_relu(el_i + er_j, 0.2); masked by adj; softmax over j; aggregate.

Key identity used on-chip: exp(lrelu(s)) = max(e^s, e^{0.2 s}).  Factoring the
per-row constant e^{0.2 el_i} (cancels in the softmax) gives attention weights
  B[j, i] = adj[i, j] * max(R_i * Er_j, Er5_j)
with R = e^{0.8 el}, Er = e^{er}, Er5 = e^{0.2 er}.  The whole N^2 x H map is
then ONE custom DVE instruction per (j-tile, head):
  ball = max(r_bc * Er, Er5) * adjT
a runtime-registered fused op (weights + adjacency mask in a single pass)
with a hand-authored 2x_1p uop variant that processes two packed fp16
elements per cycle (lo on ALU stages 0-2, hi on stages 3-5) -- 2x the
stock-op throughput of the tensor_scalar + tensor_tensor pair it replaces.
TensorE matmuls aggregate numerator and denominator (ones-column trick).

Layout: everything runs transposed ([feature/j on partitions, i on free]).
Adjacency is pre-transposed and cast to fp16 on the host into the exact
[j-on-partitions, i-on-free] tile layout the mask op wants, so there are no
on-chip transposes.  Per-iteration tiles are double-buffered, the timing
loop body is unrolled 2x, and the second body's DVE-gating head ops (r_bc,
er exponentials) are emitted before the first body's epilogue so consecutive
iterations pipeline across the For_i all-engine barrier.  The final output
is produced as out^T (host transposes back).
"""

import numpy as np

N = 4096
IN_F = 128
H = 4
F = 64
NH = H * F  # 256
OUT = 128
NCORES = 8
UNROLL = 2  # reps per For_i iteration
ROWS = N // NCORES  # 512 rows per core
JT = N // 128  # 32 j-tiles
GBLK = H * (F + 1)  # 260: g block per j-tile (64 feats + ones col per head)

_CACHE = {}


def _get_masked_attn_op():
    """Register (once) the fused DVE op  out = max(in0*s0, s1) * in1  with a
    hand-authored 2x_1p uop variant (two packed fp16 elements per cycle:
    lo on ALU stages 0-2, hi on stages 3-5)."""
    if "op" in _CACHE:
        return _CACHE["op"]
    from concourse.dve_spec import Spec, Src0, Src1, C0, C1, maxx, lower, AluOp
    from concourse.dve_ops import (
        DveOp, OPS, CUSTOM_DVE_SPECS, _SUB_OPCODE_FOR_NAME, _CUSTOM_DVE_ROW_BASE,
    )
    from concourse.dve_uop import (
        UopConfig, UopDpConfig, InpSel, OutSel, AluInp, OutPath, Trigger,
        DelayInp, DveOpSpec,
    )

    spec = Spec(
        body=maxx(Src0 * C0, C1) * Src1,
        reference=lambda in0, in1, s0, s1, imm2: (
            np.maximum(in0.astype(np.float32) * s0, s1) * in1
        ),
    )

    def build_2x_uop():
        u = UopConfig()
        u.enable_input(InpSel.SRC_0, 0)     # lane0 -> block0 ALU
        u.enable_input(InpSel.CONST_0, 1)   # d0 = s0 (Er)
        u.enable_input(InpSel.CONST_1, 2)   # d1 = s1 (Er5)
        u.enable_input(InpSel.SRC_1, 3)     # d2 = in1 lo (adj)
        u.enable_input(InpSel.SRC_0_HI, 4)  # d3 = in0 hi
        u.enable_input(InpSel.SRC_1_HI, 5)  # d4 = in1 hi
        dp = u.datapath_config
        dp[0] = UopDpConfig().enable_alu(
            AluOp.MULTIPLY, AluInp.PREV_ALU_OUT, AluInp.PREV_DELAY_0
        ).pass_through_delay(0, 1, 2, 3, 4)
        dp[1] = UopDpConfig().enable_alu(
            AluOp.MAX, AluInp.PREV_ALU_OUT, AluInp.PREV_DELAY_1
        ).pass_through_delay(0, 1, 2, 3, 4)
        dp[2] = UopDpConfig().enable_alu(
            AluOp.MULTIPLY, AluInp.PREV_ALU_OUT, AluInp.PREV_DELAY_2
        ).pass_through_delay(0, 1, 3, 4)
        dp[3] = UopDpConfig().enable_alu(
            AluOp.MULTIPLY, AluInp.PREV_DELAY_3, AluInp.PREV_DELAY_0
        ).pass_through_delay(0, 1, 4).enable_delay_from_src(DelayInp.PREV_ALU_OUT, 5)
        dp[4] = UopDpConfig().enable_alu(
            AluOp.MAX, AluInp.PREV_ALU_OUT, AluInp.PREV_DELAY_1
        ).pass_through_delay(4, 5)
        dp[5] = UopDpConfig().enable_alu(
            AluOp.MULTIPLY, AluInp.PREV_ALU_OUT, AluInp.PREV_DELAY_4
        ).pass_through_delay(5)
        dp[6] = UopDpConfig().pass_through_alu().pass_through_delay(5)
        dp[7] = UopDpConfig().pass_through_alu().pass_through_delay(5)
        u.require_inp0 = 1
        u.require_inp1 = 1
        u.trigger = (Trigger.SRC_TENSOR_DONE, Trigger.NONE, Trigger.NONE)
        u.next_uop = (0, 0, 0)
        u.enable_output(OutSel.DELAY_5, OutPath.WR0_LO)
        u.enable_output(OutSel.ALU_OUT, OutPath.WR0_HI)
        return u

    class DveOp2x(DveOp):
        def compile(self, ver):
            from concourse.dve_ops import get_dve_sub_opcode
            return DveOpSpec(
                name=self.name,
                opcode=get_dve_sub_opcode(self.name),
                uops=lower(self.spec, ver=ver),
                uops_2x=[build_2x_uop()],
                rd1_en=True,
                perf_max=1,
            )

    name = "MASKED_ATTN_W"
    if name not in _SUB_OPCODE_FOR_NAME:
        op = DveOp2x(name, spec, subdim=False, uops_sha={})
        OPS.append(op)
        CUSTOM_DVE_SPECS[name] = op.spec
        _SUB_OPCODE_FOR_NAME[name] = _CUSTOM_DVE_ROW_BASE + len(OPS) - 1
    else:
        op = next(o for o in OPS if o.name == name)
    _CACHE["op"] = op
    return op


def _build(reps=1, loop_n=None, deep=5):
    import os as _os0
    import concourse.bass as bass
    import concourse.tile as tile
    from concourse import bacc, mybir
    from concourse.masks import make_identity
    from contextlib import ExitStack

    dt = mybir.dt
    Alu = mybir.AluOpType
    Act = mybir.ActivationFunctionType

    unroll = int(_os0.environ.get("UNROLL", str(UNROLL)))
    deep = int(_os0.environ.get("DEEP", str(deep)))
    _hints = _os0.environ.get("HINTS", "0") == "1"
    nc = bacc.Bacc("TRN2", target_bir_lowering=False, debug=False)

    xT_d = nc.dram_tensor("xT", [IN_F, N], dt.float16, kind="ExternalInput").ap()
    sw_d = nc.dram_tensor("sw", [IN_F, ROWS + 2 * H + NH], dt.float16, kind="ExternalInput").ap()
    wout_d = nc.dram_tensor("wout", [F, H, OUT], dt.float16, kind="ExternalInput").ap()
    bout_d = nc.dram_tensor("bout", [1, OUT], dt.float16, kind="ExternalInput").ap()
    # adjacency, host-transposed: adjT[p, t*ROWS + i] = adj[i_row, 128*t + p]
    adjT_d = nc.dram_tensor("adjT", [128, JT * ROWS], dt.float16, kind="ExternalInput").ap()
    ind4_d = nc.dram_tensor("ind4", [H, H * 128], dt.float16, kind="ExternalInput").ap()
    out_d = nc.dram_tensor("outT", [OUT, ROWS], dt.float32, kind="ExternalOutput").ap()

    NG = 4  # er psum groups
    GJT = JT // NG  # 8 j-tiles per er group

    with tile.TileContext(nc) as tc:
        with ExitStack() as ctx:
            singles = ctx.enter_context(tc.tile_pool(name="singles", bufs=1))
            psum_acc = ctx.enter_context(tc.tile_pool(name="pacc", bufs=1, space="PSUM"))
            psum_g = ctx.enter_context(tc.tile_pool(name="pg_pool", bufs=1, space="PSUM"))
            psum_t = ctx.enter_context(tc.tile_pool(name="pt_pool", bufs=2, space="PSUM"))
            psum_er = ctx.enter_context(tc.tile_pool(name="per_pool", bufs=1, space="PSUM"))
            b_pool = ctx.enter_context(tc.tile_pool(name="bp", bufs=deep))
            dbl = ctx.enter_context(tc.tile_pool(name="dbl", bufs=2))
            ep_pool = ctx.enter_context(tc.tile_pool(name="epp", bufs=1))

            ones_row = singles.tile([1, ROWS], dt.float16)
            nc.gpsimd.memset(ones_row, 1.0)
            ones_col = singles.tile([1, 128], dt.float16)
            nc.gpsimd.memset(ones_col, 1.0)
            onesH16 = singles.tile([128, H], dt.float16)
            nc.gpsimd.memset(onesH16, 1.0)

            # ---- one-time loads ----
            sw = singles.tile([IN_F, ROWS + 2 * H + NH], dt.float16)
            nc.sync.dma_start(sw, sw_d)
            xTo = sw[:, 0:ROWS]
            wr = sw[:, ROWS : ROWS + H]
            wl = sw[:, ROWS + H : ROWS + 2 * H]
            w1 = sw[:, ROWS + 2 * H : ROWS + 2 * H + NH]
            xT = singles.tile([IN_F, N], dt.float16)
            for xc in range(4):
                nc.sync.dma_start(
                    xT[:, (N // 4) * xc : (N // 4) * (xc + 1)],
                    xT_d[:, (N // 4) * xc : (N // 4) * (xc + 1)],
                )
            wout = singles.tile([F, H, OUT], dt.float16)
            nc.sync.dma_start(wout, wout_d)
            bout = singles.tile([1, OUT], dt.float16)
            nc.sync.dma_start(bout, bout_d)
            # ind4[c, 128h+p] = 1 if c == h else 0: head-select lhsT for pbc
            ind4 = singles.tile([H, H * 128], dt.float16)
            nc.sync.dma_start(ind4, ind4_d)
            ident16 = singles.tile([128, 128], dt.float16)
            make_identity(nc, ident16)

            S = {}

            def head_a(rep):
                # ---- interleaved input DMAs: xT chunk c feeds er-group c's
                # matmuls; adjT chunk c feeds that group's mask TTs ----
                adjT = dbl.tile([128, JT * ROWS], dt.float16,
                                name=f"adjT_{rep}", tag="adjT")
                for s in range(8):
                    nc.sync.dma_start(
                        adjT[:, 4 * ROWS * s : 4 * ROWS * (s + 1)],
                        adjT_d[:, 4 * ROWS * s : 4 * ROWS * (s + 1)],
                    )

                # ---- own-row head projections: R = exp(0.8 * el), broadcast.
                # One matmul + one exp for all 4 heads, then per-head broadcast.
                pel_all = psum_g.tile([H, ROWS], dt.float32, tag="pg", name=f"pel_{rep}")
                nc.tensor.matmul(pel_all, lhsT=wl, rhs=xTo, start=True, stop=True)
                r_all = ep_pool.tile([H, ROWS], dt.float16, tag="r_all", name=f"r_all_{rep}")
                nc.scalar.activation(r_all, pel_all, Act.Exp, scale=0.8)
                r_bc = []
                for h in range(H):
                    hp_pool, hp_tag = (psum_g, "pg") if h % 2 == 0 else (psum_t, "pT")
                    pbc = hp_pool.tile([128, ROWS], dt.float32, tag=hp_tag, name=f"pbc{h}_{rep}")
                    nc.tensor.matmul(
                        pbc, lhsT=ind4[:, 128 * h : 128 * (h + 1)], rhs=r_all,
                        start=True, stop=True,
                    )
                    rb = dbl.tile([128, ROWS], dt.float16, name=f"r_bc{h}_{rep}",
                                  tag=f"r_bc{h}")
                    if h % 2 == 0:
                        nc.scalar.copy(rb, pbc)
                    else:
                        nc.vector.tensor_copy(rb, pbc)
                    r_bc.append(rb)

                # ---- er head projections (packed psum groups) + exp ----
                er_g, er5_g = [], []
                for grp in range(NG):
                    per = psum_t.tile(
                        [128, H * GJT], dt.float32, tag="pT", name=f"per{grp}_{rep}"
                    )
                    for k in range(GJT):
                        jt = GJT * grp + k
                        nc.tensor.matmul(
                            per[:, H * k : H * (k + 1)],
                            lhsT=xT[:, 128 * jt : 128 * (jt + 1)],
                            rhs=wr,
                            start=True,
                            stop=True,
                        )
                    e1 = dbl.tile([128, H * GJT], dt.float32, name=f"er_{grp}_{rep}",
                                  tag=f"er_{grp}")
                    nc.scalar.activation(e1, per, Act.Exp)
                    e5 = dbl.tile([128, H * GJT], dt.float32, name=f"er5_{grp}_{rep}",
                                  tag=f"er5_{grp}")
                    nc.scalar.activation(e5, per, Act.Exp, scale=0.2)
                    er_g.append(e1)
                    er5_g.append(e5)

                S[rep] = dict(adjT=adjT, r_bc=r_bc, er_g=er_g, er5_g=er5_g)

            def g_one(rep, jt):
                # projection g = x @ W1 for one j-tile
                pg = psum_g.tile([128, NH], dt.float32, tag="pg", name=f"pg{jt}_{rep}")
                nc.tensor.matmul(
                    pg,
                    lhsT=xT[:, 128 * jt : 128 * (jt + 1)],
                    rhs=w1,
                    start=True,
                    stop=True,
                )
                gt = dbl.tile([128, GBLK], dt.float16, name=f"g_{jt}_{rep}",
                              tag=f"g_{jt}")
                gt3 = gt.rearrange("p (h f) -> p h f", h=H)
                nc.scalar.copy(
                    gt3[:, :, 0:F], pg.rearrange("p (h f) -> p h f", h=H)
                )
                nc.scalar.copy(gt3[:, :, F : F + 1], onesH16.unsqueeze(2))
                S[rep]["g_t"][jt] = gt

            def head_b(rep):
                # first 4 j-tiles' g upfront; the rest interleave into the
                # j-loop as PE filler (keeps the PE pstate ramped between
                # aggregation bursts).
                S[rep]["g_t"] = {}
                for jt in range(4):
                    g_one(rep, jt)

            def jloop(rep):
                adjT, r_bc = S[rep]["adjT"], S[rep]["r_bc"]
                er_g, er5_g, g_t = S[rep]["er_g"], S[rep]["er5_g"], S[rep]["g_t"]
                # ---- attention accumulation, out[i-block, (f|den)] layout:
                # ball 128x128 blocks are the lhsT (128 weight cols -> FWL
                # fast-weight-load engages) and the output fills all 128
                # partitions, which keeps the PE HAM clock governor at full
                # speed (the old [65, 512] layout ran half-empty at 1.2 GHz).
                # g (with its ones column -> denominator) streams as rhs.
                pacc = [
                    psum_acc.tile([128, 4 * (F + 1)], dt.float32, name=f"acc{h}_{rep}", tag=f"acc{h}")
                    for h in range(H)
                ]
                mop = _get_masked_attn_op()

                def agg_pair(jp, balls):
                    ball = balls[jp]
                    for t in range(2):
                        jt = 2 * jp + t
                        for h in range(H):
                            for ib in range(4):
                                # start=True only on the bank's very first
                                # matmul: its whole-bank has_written clear makes
                                # the sibling ib chains overwrite on first touch.
                                nc.tensor.matmul(
                                    pacc[h][:, (F + 1) * ib : (F + 1) * (ib + 1)],
                                    lhsT=ball[:, H * ROWS * t + ROWS * h + 128 * ib : H * ROWS * t + ROWS * h + 128 * (ib + 1)],
                                    rhs=g_t[jt][:, (F + 1) * h : (F + 1) * (h + 1)],
                                    start=(jt == 0 and ib == 0),
                                    stop=(jt == JT - 1),
                                    skip_group_check=True,
                                )

                # aggregation lags the fused ops by one pair so the PE always
                # has a ready backlog (keeps it executing continuously -> full
                # pstate clock instead of oscillating at 1.2 GHz).
                balls = {}
                for jp in range(JT // 2):
                    for jt_g in (2 * jp + 4, 2 * jp + 5):
                        if jt_g < JT:
                            g_one(rep, jt_g)
                    # fused per (tile, head): ball = max(r_bc*Er, Er5) * adjT
                    # (custom DVE op, 2 packed fp16 elems/cycle)
                    ball = b_pool.tile([128, 2 * H * ROWS], dt.float16, tag="ball")
                    balls[jp] = ball
                    for t in range(2):
                        jt = 2 * jp + t
                        grp, gk = jt // GJT, jt % GJT
                        for h in range(H):
                            bi = nc.vector._custom_dve(
                                mop,
                                out=ball[:, H * ROWS * t + ROWS * h : H * ROWS * t + ROWS * (h + 1)],
                                in0=r_bc[h],
                                in1=adjT[:, jt * ROWS : (jt + 1) * ROWS],
                                s0=er_g[grp][:, H * gk + h : H * gk + h + 1],
                                s1=er5_g[grp][:, H * gk + h : H * gk + h + 1],
                            )
                            bi.ins.perf_max = 1
                    if jp >= 2:
                        agg_pair(jp - 2, balls)
                agg_pair(JT // 2 - 2, balls)
                agg_pair(JT // 2 - 1, balls)

                S[rep]["pacc"] = pacc

            def epilogue(rep):
                pacc = S[rep]["pacc"]
                # Stage the accumulators to SBUF immediately with four wide Act
                # copies: this releases the pacc PSUM banks for the next rep's
                # aggregation ~25us earlier than letting the serialized elu
                # chain below read PSUM directly (the next jloop's first agg
                # matmul WAR-blocks on the last pacc reader).
                accs = []
                for h in range(H):
                    a = ep_pool.tile([128, 4 * (F + 1)], dt.float32, tag=f"accs{h}", name=f"accs{h}_{rep}")
                    nc.scalar.copy(a, pacc[h])
                    accs.append(a)
                p3 = [accs[h].rearrange("p (ib c) -> p ib c", c=F + 1) for h in range(H)]
                # denominators live in column F of every (h, ib) block; their
                # reciprocal becomes a per-i-partition Act scale.
                den_sb = ep_pool.tile([128, H * 4], dt.float32, tag="den_sb")
                for h in range(H):
                    if h % 2 == 0:
                        nc.scalar.copy(den_sb[:, 4 * h : 4 * (h + 1)], p3[h][:, :, F])
                    else:
                        nc.vector.tensor_copy(den_sb[:, 4 * h : 4 * (h + 1)], p3[h][:, :, F])
                rec = ep_pool.tile([128, H * 4], dt.float32, tag="rec")
                nc.vector.reciprocal(rec, den_sb)

                # elu'(x) = relu(x) + min(exp(x), 1) with x = numer * rec; rec>0
                # commutes with relu, and x <= ~5 so fp16 exp cannot overflow.
                # Unique tiles per (h, ib) break WAR chains in the pipeline.
                pout = psum_er.tile([OUT, ROWS], dt.float32, tag="per_out", name=f"pout_{rep}")
                for h in range(H):
                    for ib in range(4):
                        rc = rec[:, 4 * h + ib : 4 * h + ib + 1]
                        elup = ep_pool.tile([128, F], dt.float16, tag=f"elup{h}{ib}", name=f"elup{h}_{ib}_{rep}")
                        nc.scalar.activation(elup, p3[h][:, ib, 0:F], Act.Relu, scale=rc)
                        texp = ep_pool.tile([128, F], dt.float16, tag=f"texp{h}{ib}", name=f"texp{h}_{ib}_{rep}")
                        nc.scalar.activation(texp, p3[h][:, ib, 0:F], Act.Exp, scale=rc)
                        eluh = ep_pool.tile([128, F], dt.float16, tag=f"eluh{h}{ib}", name=f"eluh{h}_{ib}_{rep}")
                        nc.vector.scalar_tensor_tensor(eluh, texp, 1.0, elup, Alu.min, Alu.add)
                        eT_p = psum_t.tile([F, 128], dt.float16, tag="pT", name=f"eT{h}_{ib}_{rep}")
                        nc.tensor.transpose(eT_p, eluh, ident16)
                        eT = ep_pool.tile([F, 128], dt.float16, tag=f"eT{h}{ib}", name=f"eTs{h}_{ib}_{rep}")
                        if ib % 2 == 0:
                            nc.scalar.copy(eT, eT_p)
                        else:
                            nc.vector.tensor_copy(eT, eT_p)
                        nc.tensor.matmul(
                            pout[:, 128 * ib : 128 * (ib + 1)],
                            lhsT=wout[:, h, :],
                            rhs=eT,
                            start=(h == 0 and ib == 0),
                            stop=False,
                            skip_group_check=True,
                        )
                nc.tensor.matmul(pout, lhsT=bout, rhs=ones_row, start=False, stop=True,
                                 skip_group_check=True)
                osb = ep_pool.tile([OUT, ROWS], dt.float32, tag="osb")
                nc.scalar.copy(osb, pout)
                nc.sync.dma_start(out_d, osb)

            import os as _os2
            _skip_epi = _os2.environ.get("SKIP_EPI", "0") == "1"
            _skip_jl = _os2.environ.get("SKIP_JL", "0") == "1"

            def rep_body(rep):
                head_a(rep)
                head_b(rep)
                if not _skip_jl:
                    jloop(rep)
                    if not _skip_epi:
                        epilogue(rep)

            if loop_n is not None:
                import os as _os
                _sr = _os.environ.get("STAG_RESET", "0") == "1"
                if loop_n == 1:
                    rep_body(0)
                else:
                    _he = tuple(mybir.ALL_ENGINES) if _hints else ()
                    with tc.For_i(0, loop_n // unroll, 1, staggered_reset=_sr,
                                  hint_engines=_he):
                        head_a(0)
                        head_b(0)
                        if not _skip_jl:
                            jloop(0)
                        for k in range(1, unroll):
                            head_a(k)
                            if not (_skip_jl or _skip_epi):
                                epilogue(k - 1)
                            head_b(k)
                            if not _skip_jl:
                                jloop(k)
                        if not (_skip_jl or _skip_epi):
                            epilogue(unroll - 1)
            else:
                for rep in range(reps):
                    rep_body(rep)

    nc.compile()
    return nc


def _prep_inputs(x, adj_mat, W1, attn_l, attn_r, W_out, b_out):
    x = np.asarray(x, dtype=np.float32)
    W1 = np.asarray(W1, dtype=np.float32)
    attn_l = np.asarray(attn_l, dtype=np.float32)
    attn_r = np.asarray(attn_r, dtype=np.float32)
    W_out = np.asarray(W_out, dtype=np.float32)
    b_out = np.asarray(b_out, dtype=np.float32)
    adj = np.asarray(adj_mat).reshape(N, N)

    xT = np.ascontiguousarray(x.T).astype(np.float16)  # [128, 4096]
    W1h = W1.reshape(IN_F, H, F)
    wr = np.einsum("ihf,f->ih", W1h, attn_r).astype(np.float16)  # [128, 4]
    wl = np.einsum("ihf,f->ih", W1h, attn_l).astype(np.float16)  # [128, 4]
    w1_16 = W1.astype(np.float16)
    wout16 = np.ascontiguousarray(W_out.reshape(H, F, OUT).transpose(1, 0, 2)).astype(
        np.float16
    )
    beff = (b_out - W_out.sum(axis=0)).astype(np.float16).reshape(1, OUT)

    adj16 = adj.astype(np.float16)  # 0/1 exact in fp16
    in_maps = []
    for c in range(NCORES):
        rows = slice(c * ROWS, (c + 1) * ROWS)
        sw = np.concatenate([xT[:, rows], wr, wl, w1_16], axis=1)
        # adjT[p, t*ROWS + i] = adj[row_i, 128*t + p]
        adjT = (
            adj16[rows]                       # [512, 4096]
            .T.reshape(JT, 128, ROWS)         # [32, 128, 512]
            .transpose(1, 0, 2)               # [128, 32, 512]
            .reshape(128, JT * ROWS)
        )
        in_maps.append(
            {
                "xT": xT,
                "sw": np.ascontiguousarray(sw),
                "wout": wout16,
                "bout": beff,
                "adjT": np.ascontiguousarray(adjT),
                "ind4": np.kron(np.eye(H), np.ones((1, 128))).astype(np.float16),
            }
        )
    return in_maps


def kernel(**inputs):
    from concourse import bass_utils

    if "nc" not in _CACHE:
        _CACHE["nc"] = _build()
    nc = _CACHE["nc"]
    in_maps = _prep_inputs(**inputs)
    res = bass_utils.run_bass_kernel_spmd(nc, in_maps, core_ids=list(range(NCORES)))
    out = np.concatenate([res.results[c]["outT"].T for c in range(NCORES)], axis=0)
    return out.astype(np.float32)

